# revision 21
# baseline (speedup 1.0000x reference)
"""ExpertGNN (2x GCN + GAT + pool + fc) on 8 trn2 cores — v2.

Sharding: dst-node blocks of 128 slots balanced by in-degree; 20 blocks/core.
Phase 1 messages (xs[src]) are host-pre-gathered (input-derived only), so no
device gathers in phase 1. Phases 2/3 gather via indirect DMA (128 rows per
call). Aggregations are one-hot matmuls in bf16 (one-hots built once via
TensorScalarPtr is_equal and cached in SBUF across phases; GAT pass-B one-hots
are built fused with the exp-logit scale). The GAT softmax denominator is
applied post-aggregation (alpha = exts * r[dst] factors through the one-hot).
t2 table packs h2 (bf16) + als (f32) into 72-f32 rows so phase 3 needs one
table. AllGathers: t1, t2; AllReduce: pooled.
"""

import numpy as np

import concourse.bass as bass
import concourse.bacc as bacc
import concourse.mybir as mybir
import concourse.tile as tile

F32 = mybir.dt.float32
BF16 = mybir.dt.bfloat16
I32 = mybir.dt.int32
AX = mybir.AxisListType
ALU = mybir.AluOpType
ACT = mybir.ActivationFunctionType

NEG_SLOPE = 0.2
EPS = 1e-16


def _bf16(x):
    import jax.numpy as jnp
    return np.asarray(jnp.asarray(np.asarray(x, np.float32), dtype=jnp.bfloat16))


# ---------------------------------------------------------------- host prep

def _pack_dvt(dstvT_core, nblk, tpb):
    """[nblk, tpb, 128] -> [128, ceil(nblk*tpb/128)*128]; row (b,t) lives at
    partition (b*tpb+t)%128, cols ((b*tpb+t)//128)*128 +: 128."""
    ndvt = (nblk * tpb + 127) // 128
    out = np.full((128, ndvt * 128), -1.0, np.float32)
    for b in range(nblk):
        for t in range(tpb):
            j = b * tpb + t
            out[j % 128, (j // 128) * 128:(j // 128 + 1) * 128] = dstvT_core[b, t]
    return out


def prep(x, edge_index, n_cores=8):
    n = x.shape[0]
    f_in = x.shape[1]
    src0 = np.asarray(edge_index[0], dtype=np.int64)
    dst0 = np.asarray(edge_index[1], dtype=np.int64)
    indeg = np.bincount(dst0, minlength=n).astype(np.int64) + 1

    nblk_total = n_cores * int(np.ceil(n / 128.0 / n_cores))
    while nblk_total * 128 < n:
        nblk_total += n_cores
    nblk = nblk_total // n_cores

    import heapq
    heap = [(0, 0, b) for b in range(nblk_total)]
    heapq.heapify(heap)
    order = np.argsort(-indeg, kind="stable")
    slot_of_node = np.empty(n, dtype=np.int64)
    block_nodes = [[] for _ in range(nblk_total)]
    for nd in order:
        load, cnt, b = heapq.heappop(heap)
        slot_of_node[nd] = b * 128 + cnt
        block_nodes[b].append(nd)
        if cnt + 1 < 128:
            heapq.heappush(heap, (load + int(indeg[nd]), cnt + 1, b))

    s_slot = slot_of_node[src0]
    d_slot = slot_of_node[dst0]
    d_blk = d_slot >> 7

    order_e = np.lexsort((s_slot, d_blk))
    s_slot = s_slot[order_e]
    d_slot = d_slot[order_e]
    d_blk = d_blk[order_e]
    starts = np.searchsorted(d_blk, np.arange(nblk_total))
    ends = np.searchsorted(d_blk, np.arange(nblk_total), side="right")
    counts = ends - starts
    tpb = int(np.ceil(counts.max() / 128.0))
    cap = tpb * 128

    # dinv and xs table (pre-scaled by dinv[src]); bf16 once, globally
    dinv_n = 1.0 / np.sqrt(indeg.astype(np.float64))
    nslots = nblk_total * 128
    dinv_slot = np.zeros(nslots, np.float32)
    dinv_slot[slot_of_node] = dinv_n.astype(np.float32)
    xs = np.zeros((nslots, f_in), np.float32)
    xs[slot_of_node] = np.asarray(x, np.float32) * dinv_n[:, None].astype(np.float32)
    xs_bf = _bf16(xs)

    ee = np.arange(cap)
    # per-block edge arrays in [e%128, e//128] layout
    gidx = np.zeros((nblk_total, 128, tpb), dtype=np.int32)
    dstv = np.full((nblk_total, 128, tpb), -1.0, dtype=np.float32)
    dstvT = np.full((nblk_total, tpb, 128), -1.0, dtype=np.float32)
    msg1 = np.zeros((nblk_total, 128, tpb * f_in), dtype=xs_bf.dtype)
    for b in range(nblk_total):
        cnt = counts[b]
        gs = np.zeros(cap, dtype=np.int32)
        gd = np.full(cap, -1.0, dtype=np.float32)
        gs[:cnt] = s_slot[starts[b]:ends[b]]
        gd[:cnt] = (d_slot[starts[b]:ends[b]] & 127).astype(np.float32)
        gidx[b, ee % 128, ee // 128] = gs
        dstv[b, ee % 128, ee // 128] = gd
        dstvT[b, ee // 128, ee % 128] = gd
        # edge j -> partition j%128, tile j//128
        m = xs_bf[gs]                     # [cap, f_in] bf16
        msg1[b] = m.reshape(tpb, 128, f_in).transpose(1, 0, 2) \
            .reshape(128, tpb * f_in)

    # own xs blocks, validity
    xsb = np.zeros((nblk_total, 128, f_in), dtype=xs_bf.dtype)
    vm = np.zeros((nblk_total, 128), np.float32)
    for b in range(nblk_total):
        nds = block_nodes[b]
        if nds:
            sl = [slot_of_node[nd] for nd in nds]
            xsb[b, :len(nds), :] = xs_bf[sl]
            vm[b, :len(nds)] = 1.0

    dinv_blk = dinv_slot.reshape(nblk_total, 128)

    meta = dict(nblk=nblk, tpb=tpb, nblk_total=nblk_total,
                nslots=nslots, n=n)
    per_core = []
    for c in range(n_cores):
        sl = slice(c * nblk, (c + 1) * nblk)
        # device layouts: partition-major where possible
        pc = dict(
            msg1=np.ascontiguousarray(
                msg1[sl].transpose(1, 0, 2).reshape(128, nblk * tpb * f_in)),
            xsb=np.ascontiguousarray(
                xsb[sl].transpose(1, 0, 2).reshape(128, nblk * f_in)),
            gidx=np.ascontiguousarray(
                gidx[sl].transpose(1, 0, 2).reshape(128, nblk * tpb)),
            dv=np.ascontiguousarray(
                dstv[sl].transpose(1, 0, 2).reshape(128, nblk * tpb)),
            dinv=np.ascontiguousarray(dinv_blk[sl].T.copy()),   # [128, nblk]
            vm=_bf16(np.ascontiguousarray(vm[sl].T.copy())),    # [128, nblk]
        )
        per_core.append(pc)
    return per_core, meta


# ------------------------------------------------------------ device program

def build(meta, weights_shapes, n_cores=8, n_queues=4,
          timing_repeats=0, scratch=65536):
    nblk = meta["nblk"]
    tpb = meta["tpb"]
    nslots = meta["nslots"]
    n_real = meta["n"]
    f_in = weights_shapes["f_in"]     # 128
    d1 = weights_shapes["d1"]         # 64
    d2 = weights_shapes["d2"]         # 128
    h_heads = weights_shapes["h"]     # 8
    f_gat = weights_shapes["f"]       # 128
    hf = h_heads * f_gat              # 1024
    ncls = weights_shapes["ncls"]     # 10
    own = nblk * 128
    t2w = d2 // 2 + h_heads           # 72 f32 words: h2 bf16 (64) + als (8)

    nc = bacc.Bacc("TRN2", target_bir_lowering=False, debug=False,
                   num_devices=n_cores, num_swdge_queues=max(1, n_queues),
                   dynamic_dma_scratch_size=scratch)

    def inp(name, shape, dt=F32):
        return nc.dram_tensor(name, shape, dt, kind="ExternalInput")

    msg1 = inp("msg1", [128, nblk * tpb * f_in], BF16)
    xsb = inp("xsb", [128, nblk * f_in], BF16)
    gidx = inp("gidx", [128, nblk * tpb], I32)
    dv_in = inp("dv", [128, nblk * tpb])
    dinv_in = inp("dinv", [128, nblk])
    vm_in = inp("vm", [128, nblk], BF16)
    w1 = inp("w1", [f_in, d1])
    b1r = inp("b1r", [128, d1])
    w2 = inp("w2", [d1, d2])
    b2r = inp("b2r", [128, d2])
    wgbf = inp("wgbf", [d2, hf], BF16)
    awad = inp("awad", [d2, 2 * h_heads], BF16)
    bgrow = inp("bgrow", [1, hf], BF16)
    wfc = inp("wfc", [hf, ncls])
    bfc = inp("bfc", [ncls, 1])
    out = nc.dram_tensor("out", [ncls], F32, kind="ExternalOutput")

    def shared(name, shape, dt=F32):
        return nc.dram_tensor(name, shape, dt, kind="Internal",
                              addr_space="Shared")

    t1_own = nc.dram_tensor("t1_own", [own, f_in], BF16, kind="Internal")
    t1_full = shared("t1_full", [nslots, f_in], BF16)
    t2_own = nc.dram_tensor("t2_own", [own, t2w], F32, kind="Internal")
    t2_full = shared("t2_full", [nslots, t2w], F32)
    pool_in = nc.dram_tensor("pool_in", [1, hf], F32, kind="Internal")
    pool_out = shared("pool_out", [1, hf])
    scratch_d = nc.dram_tensor("scratch", [1, max(hf, 32)], F32,
                               kind="Internal")

    rg = [list(range(n_cores))]
    q_counter = [0]

    def gather(dst_tile_ap, table_ap, idx_ap):
        inst = nc.gpsimd.indirect_dma_start(
            out=dst_tile_ap, out_offset=None, in_=table_ap,
            in_offset=bass.IndirectOffsetOnAxis(ap=idx_ap, axis=0))
        if n_queues > 1:
            qi = q_counter[0] % n_queues
            q_counter[0] += 1
            inst.ins.queue = f"qPoolDynamic{qi or ''}"
        return inst

    with tile.TileContext(nc) as tc:
        with tc.tile_pool(name="const", bufs=1) as constp, \
             tc.tile_pool(name="cache", bufs=1) as cachep, \
             tc.tile_pool(name="src", bufs=2) as srcp, \
             tc.tile_pool(name="src3", bufs=3) as src3p, \
             tc.tile_pool(name="ohh", bufs=2) as ohhp, \
             tc.tile_pool(name="blk", bufs=2) as blkp, \
             tc.tile_pool(name="small", bufs=4) as smallp, \
             tc.tile_pool(name="fin", bufs=1) as finp, \
             tc.tile_pool(name="psA", bufs=2, space="PSUM") as psA, \
             tc.tile_pool(name="psB", bufs=1, space="PSUM") as psB, \
             tc.tile_pool(name="psS", bufs=2, space="PSUM") as psS, \
             tc.tile_pool(name="psP", bufs=1, space="PSUM") as psP:

            # ---------------- constants / caches
            iota_bf = constp.tile([128, 128], BF16)
            nc.gpsimd.iota(iota_bf[:], pattern=[[1, 128]], base=0,
                           channel_multiplier=0,
                           allow_small_or_imprecise_dtypes=True)
            iota_colf = constp.tile([128, 1], F32)
            nc.gpsimd.iota(iota_colf[:], pattern=[[0, 1]], base=0,
                           channel_multiplier=1,
                           allow_small_or_imprecise_dtypes=True)
            iota_col128 = constp.tile([128, 128], BF16)
            nc.gpsimd.iota(iota_col128[:], pattern=[[0, 128]], base=0,
                           channel_multiplier=1,
                           allow_small_or_imprecise_dtypes=True)
            ident_bf = constp.tile([128, 128], BF16)
            nc.vector.tensor_tensor(out=ident_bf[:], in0=iota_col128[:],
                                    in1=iota_bf[:], op=ALU.is_equal)
            ones_row = constp.tile([1, 128], BF16)
            nc.vector.memset(ones_row[:], 1.0)

            w1sb = constp.tile([f_in, d1], F32)
            nc.sync.dma_start(out=w1sb[:], in_=w1[:])
            b1sb = constp.tile([128, d1], F32)
            nc.sync.dma_start(out=b1sb[:], in_=b1r[:])
            w2sb = constp.tile([d1, d2], F32)
            nc.sync.dma_start(out=w2sb[:], in_=w2[:])
            b2sb = constp.tile([128, d2], F32)
            nc.sync.dma_start(out=b2sb[:], in_=b2r[:])
            wgsb = constp.tile([d2, hf], BF16)
            nc.sync.dma_start(out=wgsb[:], in_=wgbf[:])
            awadsb = constp.tile([d2, 2 * h_heads], BF16)
            nc.sync.dma_start(out=awadsb[:], in_=awad[:])
            bgsb = constp.tile([1, hf], BF16)
            nc.sync.dma_start(out=bgsb[:], in_=bgrow[:])

            dinv_all = cachep.tile([128, nblk], F32)
            nc.sync.dma_start(out=dinv_all[:], in_=dinv_in[:])
            vm_all = cachep.tile([128, nblk], BF16)
            nc.sync.dma_start(out=vm_all[:], in_=vm_in[:])
            dv_all = cachep.tile([128, nblk * tpb], F32)
            nc.sync.dma_start(out=dv_all[:], in_=dv_in[:])
            gidx_all = cachep.tile([128, nblk * tpb], I32)
            nc.sync.dma_start(out=gidx_all[:], in_=gidx[:])

            oh_all = cachep.tile([128, nblk * tpb * 128], BF16)
            t1c = cachep.tile([128, nblk * f_in], BF16)
            nc.vector.memset(t1c[:], 0.0)
            t2hc = cachep.tile([128, nblk * d2], BF16)
            aac = cachep.tile([128, nblk * 2 * h_heads], F32)  # als|ald own

            def _whole_body(_i=None):
                # ---------------- phase 1: GCN1 (host-pregathered messages)
                for b in range(nblk):
                    m1 = srcp.tile([128, tpb * f_in], BF16, tag="m1")
                    nc.sync.dma_start(
                        out=m1[:],
                        in_=msg1[:, b * tpb * f_in:(b + 1) * tpb * f_in])
                    agg = psA.tile([f_in, 128], F32, space="PSUM", tag="agg")
                    for t in range(tpb):
                        o = oh_all[:, (b * tpb + t) * 128:(b * tpb + t + 1) * 128]
                        nc.vector.tensor_scalar(
                            out=o, in0=iota_bf[:],
                            scalar1=dv_all[:, b * tpb + t:b * tpb + t + 1],
                            scalar2=None, op0=ALU.is_equal)
                        nc.tensor.matmul(
                            out=agg[:], lhsT=m1[:, t * f_in:(t + 1) * f_in],
                            rhs=o, start=(t == 0), stop=False)
                    xsb_b = smallp.tile([128, f_in], BF16, tag="xsbb")
                    nc.sync.dma_start(
                        out=xsb_b[:], in_=xsb[:, b * f_in:(b + 1) * f_in])
                    nc.tensor.matmul(
                        out=agg[:], lhsT=xsb_b[:],
                        rhs=ident_bf[:], start=False, stop=True)
                    aggs = blkp.tile([f_in, 128], F32, tag="aggs")
                    nc.vector.tensor_copy(out=aggs[:], in_=agg[:])
                    h1ps = psS.tile([128, d1], F32, space="PSUM", tag="tr")
                    nc.tensor.matmul(out=h1ps[:], lhsT=aggs[:], rhs=w1sb[:],
                                     start=True, stop=True)
                    h1a = smallp.tile([128, d1], F32, tag="h1a")
                    nc.vector.tensor_scalar(
                        out=h1a[:], in0=h1ps[:],
                        scalar1=dinv_all[:, b:b + 1], scalar2=None,
                        op0=ALU.mult)
                    nc.vector.tensor_tensor(
                        out=h1a[:], in0=h1a[:],
                        in1=b1sb[:], op=ALU.add)
                    nc.scalar.activation(out=h1a[:], in_=h1a[:], func=ACT.Relu)
                    nc.vector.tensor_scalar(
                        out=t1c[:, b * f_in:b * f_in + d1], in0=h1a[:],
                        scalar1=dinv_all[:, b:b + 1], scalar2=None,
                        op0=ALU.mult)
                    nc.sync.dma_start(
                        out=t1_own[b * 128:(b + 1) * 128, :],
                        in_=t1c[:, b * f_in:(b + 1) * f_in])

                if timing_repeats:
                    nc.sync.dma_start(out=t1_full[:own, :], in_=t1_own[:])
                else:
                    nc.gpsimd.collective_compute(
                        "AllGather", ALU.bypass, replica_groups=rg,
                        ins=[t1_own[:]], outs=[t1_full[:]])

                # ---------------- phase 2: GCN2 -> t2 table [h2.bf16 | als]
                for b in range(nblk):
                    t1t = srcp.tile([128, tpb, f_in], BF16, tag="t1t")
                    for t in range(tpb):
                        gather(t1t[:, t, :], t1_full[:],
                               gidx_all[:, b * tpb + t:b * tpb + t + 1])
                    agg = psA.tile([f_in, 128], F32, space="PSUM", tag="agg")
                    for t in range(tpb):
                        o = oh_all[:, (b * tpb + t) * 128:(b * tpb + t + 1) * 128]
                        nc.tensor.matmul(out=agg[:], lhsT=t1t[:, t, :], rhs=o,
                                         start=(t == 0), stop=False)
                    nc.tensor.matmul(
                        out=agg[:], lhsT=t1c[:, b * f_in:(b + 1) * f_in],
                        rhs=ident_bf[:], start=False, stop=True)
                    aggs = blkp.tile([f_in, 128], F32, tag="aggs")
                    nc.vector.tensor_copy(out=aggs[:], in_=agg[:])
                    h2ps = psS.tile([128, d2], F32, space="PSUM", tag="tr")
                    nc.tensor.matmul(out=h2ps[:], lhsT=aggs[:d1, :],
                                     rhs=w2sb[:], start=True, stop=True)
                    h2a = blkp.tile([128, d2], F32, tag="h2a")
                    nc.vector.tensor_scalar(
                        out=h2a[:], in0=h2ps[:],
                        scalar1=dinv_all[:, b:b + 1], scalar2=None,
                        op0=ALU.mult)
                    nc.vector.tensor_tensor(
                        out=h2a[:], in0=h2a[:],
                        in1=b2sb[:], op=ALU.add)
                    h2blk = t2hc[:, b * d2:(b + 1) * d2]
                    nc.scalar.activation(out=h2blk, in_=h2a[:], func=ACT.Relu)
                    # h2T (bf16) for als/ald
                    h2Tps = psS.tile([d2, 128], BF16, space="PSUM", tag="tr")
                    nc.tensor.transpose(out=h2Tps[:], in_=h2blk,
                                        identity=ident_bf[:])
                    h2T = blkp.tile([d2, 128], BF16, tag="h2T")
                    nc.vector.tensor_copy(out=h2T[:], in_=h2Tps[:])
                    aaps = psS.tile([128, 2 * h_heads], F32, space="PSUM",
                                    tag="tr")
                    nc.tensor.matmul(out=aaps[:], lhsT=h2T[:], rhs=awadsb[:],
                                     start=True, stop=True)
                    aa = aac[:, b * 2 * h_heads:(b + 1) * 2 * h_heads]
                    nc.vector.tensor_copy(out=aa, in_=aaps[:])
                    # table row: [h2 bf16 (64 f32 words) | als (8 f32)]
                    nc.sync.dma_start(
                        out=t2_own[b * 128:(b + 1) * 128, :d2 // 2],
                        in_=h2blk.bitcast(F32))
                    nc.sync.dma_start(
                        out=t2_own[b * 128:(b + 1) * 128, d2 // 2:],
                        in_=aa[:, :h_heads])

                if timing_repeats:
                    nc.sync.dma_start(out=t2_full[:own, :], in_=t2_own[:])
                else:
                    nc.gpsimd.collective_compute(
                        "AllGather", ALU.bypass, replica_groups=rg,
                        ins=[t2_own[:]], outs=[t2_full[:]])

                # ---------------- phase 3: GAT
                pool_ps0 = psP.tile([1, hf // 2], F32, space="PSUM",
                                    tag="pool0", name="pool_ps0")
                pool_ps1 = psP.tile([1, hf // 2], F32, space="PSUM",
                                    tag="pool1", name="pool_ps1")
                pool_ps = [pool_ps0, pool_ps1]
                for b in range(nblk):
                    t2t = src3p.tile([128, tpb, t2w], F32, tag="t2t")
                    for t in range(tpb):
                        gather(t2t[:, t, :], t2_full[:],
                               gidx_all[:, b * tpb + t:b * tpb + t + 1])
                    # ---- pass A: exts for all tiles
                    aldb = aac[:, b * 2 * h_heads + h_heads:
                               (b + 1) * 2 * h_heads]
                    alde_ps = psS.tile([128, tpb, h_heads], F32, space="PSUM",
                                       tag="tr")
                    aldb_bf = smallp.tile([128, h_heads], BF16,
                                          tag="aldbbf")
                    nc.vector.tensor_copy(out=aldb_bf[:], in_=aldb)
                    for t in range(tpb):
                        o = oh_all[:, (b * tpb + t) * 128:(b * tpb + t + 1) * 128]
                        ohT_ps = psS.tile([128, 128], BF16, space="PSUM",
                                          tag="tr")
                        nc.tensor.transpose(out=ohT_ps[:], in_=o,
                                            identity=ident_bf[:])
                        ohT = smallp.tile([128, 128], BF16, tag="ohT")
                        nc.vector.tensor_copy(out=ohT[:], in_=ohT_ps[:])
                        nc.tensor.matmul(out=alde_ps[:, t, :], lhsT=ohT[:],
                                         rhs=aldb_bf[:], start=True, stop=True)
                    lg = blkp.tile([128, tpb, h_heads], F32, tag="lg")
                    nc.vector.tensor_tensor(
                        out=lg[:], in0=alde_ps[:],
                        in1=t2t[:, :, d2 // 2:], op=ALU.add)
                    lneg = blkp.tile([128, tpb, h_heads], F32, tag="lneg")
                    nc.vector.tensor_scalar(out=lneg[:], in0=lg[:],
                                            scalar1=0.0, scalar2=NEG_SLOPE,
                                            op0=ALU.min, op1=ALU.mult)
                    nc.vector.tensor_scalar(out=lg[:], in0=lg[:], scalar1=0.0,
                                            scalar2=None, op0=ALU.max)
                    nc.vector.tensor_tensor(out=lg[:], in0=lg[:], in1=lneg[:],
                                            op=ALU.add)
                    exts = blkp.tile([128, tpb, h_heads], F32, tag="exts")
                    nc.scalar.activation(out=exts[:], in_=lg[:], func=ACT.Exp)
                    exts_bf = blkp.tile([128, tpb, h_heads], BF16, tag="extsb")
                    nc.vector.tensor_copy(out=exts_bf[:], in_=exts[:])
                    # self logits
                    aa = aac[:, b * 2 * h_heads:(b + 1) * 2 * h_heads]
                    lgs = smallp.tile([128, h_heads], F32, tag="lgs")
                    nc.vector.tensor_tensor(out=lgs[:], in0=aa[:, :h_heads],
                                            in1=aa[:, h_heads:], op=ALU.add)
                    lnegs = smallp.tile([128, h_heads], F32, tag="lnegs")
                    nc.vector.tensor_scalar(out=lnegs[:], in0=lgs[:],
                                            scalar1=0.0, scalar2=NEG_SLOPE,
                                            op0=ALU.min, op1=ALU.mult)
                    nc.vector.tensor_scalar(out=lgs[:], in0=lgs[:],
                                            scalar1=0.0, scalar2=None,
                                            op0=ALU.max)
                    nc.vector.tensor_tensor(out=lgs[:], in0=lgs[:],
                                            in1=lnegs[:], op=ALU.add)
                    ex_self = smallp.tile([128, h_heads], F32, tag="exself")
                    nc.scalar.activation(out=ex_self[:], in_=lgs[:],
                                         func=ACT.Exp)
                    ex_self_bf = smallp.tile([128, h_heads], BF16,
                                             tag="exselfb")
                    nc.vector.tensor_copy(out=ex_self_bf[:], in_=ex_self[:])
                    # s per dst slot
                    s_ps = psS.tile([128, h_heads], F32, space="PSUM",
                                    tag="tr")
                    for t in range(tpb):
                        o = oh_all[:, (b * tpb + t) * 128:(b * tpb + t + 1) * 128]
                        nc.tensor.matmul(out=s_ps[:], lhsT=o,
                                         rhs=exts_bf[:, t, :],
                                         start=(t == 0), stop=False)
                    nc.tensor.matmul(out=s_ps[:], lhsT=ident_bf[:],
                                     rhs=ex_self_bf[:], start=False, stop=True)
                    rblk = smallp.tile([128, h_heads], F32, tag="rblk")
                    nc.vector.tensor_scalar(out=rblk[:], in0=s_ps[:],
                                            scalar1=EPS, scalar2=None,
                                            op0=ALU.add)
                    nc.vector.reciprocal(out=rblk[:], in_=rblk[:])
                    # ---- pass B: aggregate exts-weighted h2
                    agg3 = psB.tile([128, h_heads, 128], F32, space="PSUM",
                                    tag="agg3")
                    for t in range(tpb):
                        ohh = ohhp.tile([128, h_heads, 128], BF16, tag="ohh")
                        for hh in range(h_heads):
                            nc.vector.tensor_scalar(
                                out=ohh[:, hh, :], in0=iota_bf[:],
                                scalar1=dv_all[:, b * tpb + t:b * tpb + t + 1],
                                scalar2=exts[:, t, hh:hh + 1],
                                op0=ALU.is_equal, op1=ALU.mult)
                        h2src = t2t[:, t, :d2 // 2].bitcast(BF16)
                        for half in range(2):
                            nc.tensor.matmul(
                                out=agg3[:, half * 4:(half + 1) * 4, :],
                                lhsT=h2src,
                                rhs=ohh[:, half * 4:(half + 1) * 4, :],
                                start=(t == 0), stop=False,
                                skip_group_check=True)
                    # self contributions
                    h2self = t2hc[:, b * d2:(b + 1) * d2]
                    for hh in range(h_heads):
                        hsc = smallp.tile([128, d2], BF16, tag="hsc")
                        nc.vector.tensor_scalar(
                            out=hsc[:], in0=h2self,
                            scalar1=ex_self[:, hh:hh + 1], scalar2=None,
                            op0=ALU.mult)
                        nc.tensor.matmul(out=agg3[:, hh, :], lhsT=hsc[:],
                                         rhs=ident_bf[:], start=False,
                                         stop=(hh in (3, 7)),
                                         skip_group_check=True)
                    aggsb = blkp.tile([128, h_heads, 128], BF16, tag="agg3s")
                    nc.vector.tensor_copy(out=aggsb[:], in_=agg3[:])
                    # og = agg @ Wg per head (+ bg), then r-scale + relu
                    og = psB.tile([128, hf], F32, space="PSUM", tag="agg3")
                    for half in range(2):
                        nc.tensor.matmul(
                            out=og[:, half * (hf // 2):(half + 1) * (hf // 2)],
                            lhsT=ones_row[:],
                            rhs=bgsb[:, half * (hf // 2):(half + 1) * (hf // 2)],
                            start=True, stop=False, skip_group_check=True)
                    for hh in range(h_heads):
                        nc.tensor.matmul(
                            out=og[:, hh * f_gat:(hh + 1) * f_gat],
                            lhsT=aggsb[:, hh, :],
                            rhs=wgsb[:, hh * f_gat:(hh + 1) * f_gat],
                            start=False, stop=(hh % 4 == 3),
                            skip_group_check=True)
                    gat = blkp.tile([128, hf], BF16, tag="gat")
                    nc.vector.tensor_tensor(
                        out=gat[:].rearrange("p (h f) -> p h f", h=h_heads),
                        in0=og[:].rearrange("p (h f) -> p h f", h=h_heads),
                        in1=rblk[:].rearrange("p (h o) -> p h o", o=1)
                            .to_broadcast([128, h_heads, f_gat]),
                        op=ALU.mult)
                    nc.scalar.activation(out=gat[:], in_=gat[:], func=ACT.Relu)
                    for half in range(2):
                        nc.tensor.matmul(
                            out=pool_ps[half][:],
                            lhsT=vm_all[:, b:b + 1],
                            rhs=gat[:, half * (hf // 2):(half + 1) * (hf // 2)],
                            start=(b == 0), stop=(b == nblk - 1))

                # ---------------- phase 4: AllReduce pooled, fc, softmax
                pooled = finp.tile([1, hf], F32, tag="pooled")
                nc.vector.tensor_copy(out=pooled[:, :hf // 2],
                                      in_=pool_ps[0][:])
                nc.vector.tensor_copy(out=pooled[:, hf // 2:],
                                      in_=pool_ps[1][:])
                nc.sync.dma_start(out=pool_in[:], in_=pooled[:1, :])
                if timing_repeats:
                    nc.sync.dma_start(out=pool_out[:], in_=pool_in[:])
                else:
                    nc.gpsimd.collective_compute(
                        "AllReduce", ALU.add, replica_groups=rg,
                        ins=[pool_in[:]], outs=[pool_out[:]])
                mean = pooled
                nc.sync.dma_start(out=mean[:], in_=pool_out[:])
                nc.vector.tensor_scalar(out=mean[:], in0=mean[:],
                                        scalar1=1.0 / n_real, scalar2=None,
                                        op0=ALU.mult)
                nc.sync.dma_start(out=scratch_d[0, :hf], in_=mean[:1, :])
                fc_ps = psS.tile([ncls, 1], F32, space="PSUM", tag="tr")
                n_chunks = hf // 128
                for ci in range(n_chunks):
                    mcol = smallp.tile([128, 1], F32, tag="mcol")
                    nc.sync.dma_start(
                        out=mcol[:],
                        in_=scratch_d[0, ci * 128:(ci + 1) * 128, None])
                    wfc_sb = smallp.tile([128, ncls], F32, tag="wfcsb")
                    nc.sync.dma_start(out=wfc_sb[:],
                                      in_=wfc[ci * 128:(ci + 1) * 128, :])
                    nc.tensor.matmul(out=fc_ps[:], lhsT=wfc_sb[:], rhs=mcol[:],
                                     start=(ci == 0), stop=(ci == n_chunks - 1))
                bfc_sb = smallp.tile([ncls, 1], F32, tag="bfcsb")
                nc.sync.dma_start(out=bfc_sb[:], in_=bfc[:])
                logit = smallp.tile([ncls, 1], F32, tag="logit")
                nc.vector.tensor_tensor(out=logit[:], in0=fc_ps[:],
                                        in1=bfc_sb[:], op=ALU.add)
                nc.sync.dma_start(out=scratch_d[0, :ncls, None],
                                  in_=logit[:, :1])
                lrow = smallp.tile([1, ncls], F32, tag="lrow")
                nc.sync.dma_start(out=lrow[:], in_=scratch_d[:1, :ncls])
                erow = smallp.tile([1, ncls], F32, tag="erow")
                nc.scalar.activation(out=erow[:], in_=lrow[:], func=ACT.Exp)
                ssum = smallp.tile([1, 1], F32, tag="ssum")
                nc.vector.reduce_sum(out=ssum[:], in_=erow[:], axis=AX.X)
                nc.vector.reciprocal(out=ssum[:], in_=ssum[:])
                nc.vector.tensor_tensor(
                    out=erow[:], in0=erow[:],
                    in1=ssum[:1, :1].to_broadcast([1, ncls]), op=ALU.mult)
                nc.sync.dma_start(out=out[None, :], in_=erow[:1, :])

            if timing_repeats > 0:
                with tc.For_i(0, timing_repeats, 1) as _i:
                    _whole_body(_i)
            else:
                _whole_body()

    nc.compile()
    return nc


def make_in_maps(per_core, w):
    maps = []
    for pc in per_core:
        m = dict(pc)
        m.update(w)
        maps.append(m)
    return maps


def weights_dict(W1, b1, W2, b2, Wg, a_src, a_dst, bg, Wfc, bfc):
    Wg = np.asarray(Wg, np.float32)
    a_src = np.asarray(a_src, np.float32)
    a_dst = np.asarray(a_dst, np.float32)
    H, F = a_src.shape
    aw = np.einsum("khf,hf->kh", Wg.reshape(Wg.shape[0], H, F), a_src)
    ad = np.einsum("khf,hf->kh", Wg.reshape(Wg.shape[0], H, F), a_dst)
    return dict(
        w1=np.asarray(W1, np.float32),
        b1r=np.tile(np.asarray(b1, np.float32).reshape(1, -1), (128, 1)),
        w2=np.asarray(W2, np.float32),
        b2r=np.tile(np.asarray(b2, np.float32).reshape(1, -1), (128, 1)),
        wgbf=_bf16(Wg),
        awad=_bf16(np.concatenate([aw, ad], axis=1)),
        bgrow=_bf16(np.asarray(bg, np.float32).reshape(1, -1)),
        wfc=np.asarray(Wfc, np.float32),
        bfc=np.asarray(bfc, np.float32).reshape(-1, 1),
    )


# ------------------------------------------------------------ harness entry

_CACHE = {}


def kernel(**inputs):
    """Full-input entry: shards across 8 trn2 cores internally."""
    x = np.asarray(inputs["x"], dtype=np.float32)
    edge_index = np.asarray(inputs["edge_index"])
    n_cores = 8

    per_core, meta = prep(x, edge_index, n_cores=n_cores)
    shapes = dict(f_in=128, d1=64, d2=128, h=8, f=128, ncls=10)

    key = (meta["nblk"], meta["tpb"], meta["nslots"], meta["n"])
    if key in _CACHE:
        nc = _CACHE[key]
    else:
        nc = build(meta, shapes, n_cores=n_cores, n_queues=4)
        _CACHE[key] = nc

    wd = weights_dict(inputs["W1"], inputs["b1"], inputs["W2"],
                      inputs["b2"], inputs["Wg"], inputs["a_src"],
                      inputs["a_dst"], inputs["bg"], inputs["Wfc"],
                      inputs["bfc"])
    in_maps = make_in_maps(per_core, wd)

    from concourse.bass_utils import run_bass_kernel_spmd
    res = run_bass_kernel_spmd(nc, in_maps, core_ids=list(range(n_cores)))
    return np.asarray(res.results[0]["out"], dtype=np.float32)


# revision 22
# speedup vs baseline: 1.1769x; 1.1769x over previous
"""ExpertGNN (2x GCN + GAT + pool + fc) on 8 trn2 cores — v2.

Sharding: dst-node blocks of 128 slots balanced by in-degree; 20 blocks/core.
Phase 1 messages (xs[src]) are host-pre-gathered (input-derived only), so no
device gathers in phase 1. Phases 2/3 gather via indirect DMA (128 rows per
call). Aggregations are one-hot matmuls in bf16 (one-hots built once via
TensorScalarPtr is_equal and cached in SBUF across phases; GAT pass-B one-hots
are built fused with the exp-logit scale). The GAT softmax denominator is
applied post-aggregation (alpha = exts * r[dst] factors through the one-hot).
t2 table packs h2 (bf16) + als (f32) into 72-f32 rows so phase 3 needs one
table. AllGathers: t1, t2; AllReduce: pooled.
"""

import numpy as np

import concourse.bass as bass
import concourse.bacc as bacc
import concourse.mybir as mybir
import concourse.tile as tile

F32 = mybir.dt.float32
BF16 = mybir.dt.bfloat16
I32 = mybir.dt.int32
AX = mybir.AxisListType
ALU = mybir.AluOpType
ACT = mybir.ActivationFunctionType

NEG_SLOPE = 0.2
EPS = 1e-16


def _bf16(x):
    import jax.numpy as jnp
    return np.asarray(jnp.asarray(np.asarray(x, np.float32), dtype=jnp.bfloat16))


# ---------------------------------------------------------------- host prep

def _pack_dvt(dstvT_core, nblk, tpb):
    """[nblk, tpb, 128] -> [128, ceil(nblk*tpb/128)*128]; row (b,t) lives at
    partition (b*tpb+t)%128, cols ((b*tpb+t)//128)*128 +: 128."""
    ndvt = (nblk * tpb + 127) // 128
    out = np.full((128, ndvt * 128), -1.0, np.float32)
    for b in range(nblk):
        for t in range(tpb):
            j = b * tpb + t
            out[j % 128, (j // 128) * 128:(j // 128 + 1) * 128] = dstvT_core[b, t]
    return out


def prep(x, edge_index, n_cores=8):
    n = x.shape[0]
    f_in = x.shape[1]
    src0 = np.asarray(edge_index[0], dtype=np.int64)
    dst0 = np.asarray(edge_index[1], dtype=np.int64)
    indeg = np.bincount(dst0, minlength=n).astype(np.int64) + 1

    nblk_total = n_cores * int(np.ceil(n / 128.0 / n_cores))
    while nblk_total * 128 < n:
        nblk_total += n_cores
    nblk = nblk_total // n_cores

    import heapq
    heap = [(0, 0, b) for b in range(nblk_total)]
    heapq.heapify(heap)
    order = np.argsort(-indeg, kind="stable")
    slot_of_node = np.empty(n, dtype=np.int64)
    block_nodes = [[] for _ in range(nblk_total)]
    for nd in order:
        load, cnt, b = heapq.heappop(heap)
        slot_of_node[nd] = b * 128 + cnt
        block_nodes[b].append(nd)
        if cnt + 1 < 128:
            heapq.heappush(heap, (load + int(indeg[nd]), cnt + 1, b))

    s_slot = slot_of_node[src0]
    d_slot = slot_of_node[dst0]
    d_blk = d_slot >> 7

    order_e = np.lexsort((s_slot, d_blk))
    s_slot = s_slot[order_e]
    d_slot = d_slot[order_e]
    d_blk = d_blk[order_e]
    starts = np.searchsorted(d_blk, np.arange(nblk_total))
    ends = np.searchsorted(d_blk, np.arange(nblk_total), side="right")
    counts = ends - starts
    tpb = int(np.ceil(counts.max() / 128.0))
    cap = tpb * 128

    # dinv and xs table (pre-scaled by dinv[src]); bf16 once, globally
    dinv_n = 1.0 / np.sqrt(indeg.astype(np.float64))
    nslots = nblk_total * 128
    dinv_slot = np.zeros(nslots, np.float32)
    dinv_slot[slot_of_node] = dinv_n.astype(np.float32)
    xs = np.zeros((nslots, f_in), np.float32)
    xs[slot_of_node] = np.asarray(x, np.float32) * dinv_n[:, None].astype(np.float32)
    xs_bf = _bf16(xs)

    ee = np.arange(cap)
    # per-block edge arrays in [e%128, e//128] layout
    gidx = np.zeros((nblk_total, 128, tpb), dtype=np.int32)
    dstv = np.full((nblk_total, 128, tpb), -1.0, dtype=np.float32)
    dstvT = np.full((nblk_total, tpb, 128), -1.0, dtype=np.float32)
    msg1 = np.zeros((nblk_total, 128, tpb * f_in), dtype=xs_bf.dtype)
    for b in range(nblk_total):
        cnt = counts[b]
        gs = np.zeros(cap, dtype=np.int32)
        gd = np.full(cap, -1.0, dtype=np.float32)
        gs[:cnt] = s_slot[starts[b]:ends[b]]
        gd[:cnt] = (d_slot[starts[b]:ends[b]] & 127).astype(np.float32)
        gidx[b, ee % 128, ee // 128] = gs
        dstv[b, ee % 128, ee // 128] = gd
        dstvT[b, ee // 128, ee % 128] = gd
        # edge j -> partition j%128, tile j//128
        m = xs_bf[gs]                     # [cap, f_in] bf16
        msg1[b] = m.reshape(tpb, 128, f_in).transpose(1, 0, 2) \
            .reshape(128, tpb * f_in)

    # own xs blocks, validity
    xsb = np.zeros((nblk_total, 128, f_in), dtype=xs_bf.dtype)
    vm = np.zeros((nblk_total, 128), np.float32)
    for b in range(nblk_total):
        nds = block_nodes[b]
        if nds:
            sl = [slot_of_node[nd] for nd in nds]
            xsb[b, :len(nds), :] = xs_bf[sl]
            vm[b, :len(nds)] = 1.0

    dinv_blk = dinv_slot.reshape(nblk_total, 128)

    meta = dict(nblk=nblk, tpb=tpb, nblk_total=nblk_total,
                nslots=nslots, n=n)
    per_core = []
    for c in range(n_cores):
        sl = slice(c * nblk, (c + 1) * nblk)
        # device layouts: partition-major where possible
        pc = dict(
            msg1=np.ascontiguousarray(
                msg1[sl].transpose(1, 0, 2).reshape(128, nblk * tpb * f_in)),
            xsb=np.ascontiguousarray(
                xsb[sl].transpose(1, 0, 2).reshape(128, nblk * f_in)),
            gidx=np.ascontiguousarray(
                gidx[sl].transpose(1, 0, 2).reshape(128, nblk * tpb)),
            dv=np.ascontiguousarray(
                dstv[sl].transpose(1, 0, 2).reshape(128, nblk * tpb)),
            dinv=np.ascontiguousarray(dinv_blk[sl].T.copy()),   # [128, nblk]
            vm=_bf16(np.ascontiguousarray(vm[sl].T.copy())),    # [128, nblk]
        )
        per_core.append(pc)
    return per_core, meta


# ------------------------------------------------------------ device program

def build(meta, weights_shapes, n_cores=8, n_queues=4,
          timing_repeats=0, scratch=65536):
    nblk = meta["nblk"]
    tpb = meta["tpb"]
    nslots = meta["nslots"]
    n_real = meta["n"]
    f_in = weights_shapes["f_in"]     # 128
    d1 = weights_shapes["d1"]         # 64
    d2 = weights_shapes["d2"]         # 128
    h_heads = weights_shapes["h"]     # 8
    f_gat = weights_shapes["f"]       # 128
    hf = h_heads * f_gat              # 1024
    ncls = weights_shapes["ncls"]     # 10
    own = nblk * 128
    t2w = d2 // 2 + h_heads           # 72 f32 words: h2 bf16 (64) + als (8)

    nc = bacc.Bacc("TRN2", target_bir_lowering=False, debug=False,
                   num_devices=n_cores, num_swdge_queues=max(1, n_queues),
                   dynamic_dma_scratch_size=scratch)

    def inp(name, shape, dt=F32):
        return nc.dram_tensor(name, shape, dt, kind="ExternalInput")

    msg1 = inp("msg1", [128, nblk * tpb * f_in], BF16)
    xsb = inp("xsb", [128, nblk * f_in], BF16)
    gidx = inp("gidx", [128, nblk * tpb], I32)
    dv_in = inp("dv", [128, nblk * tpb])
    dinv_in = inp("dinv", [128, nblk])
    vm_in = inp("vm", [128, nblk], BF16)
    w1 = inp("w1", [f_in, d1])
    b1r = inp("b1r", [128, d1])
    w2 = inp("w2", [d1, d2])
    b2r = inp("b2r", [128, d2])
    wgbf = inp("wgbf", [d2, hf], BF16)
    awad = inp("awad", [d2, 2 * h_heads], BF16)
    bgrow = inp("bgrow", [1, hf], BF16)
    wfc = inp("wfc", [hf, ncls])
    bfc = inp("bfc", [ncls, 1])
    out = nc.dram_tensor("out", [ncls], F32, kind="ExternalOutput")

    def shared(name, shape, dt=F32):
        return nc.dram_tensor(name, shape, dt, kind="Internal",
                              addr_space="Shared")

    t1_own = nc.dram_tensor("t1_own", [own, f_in], BF16, kind="Internal")
    t1_full = shared("t1_full", [nslots, f_in], BF16)
    t2_own = nc.dram_tensor("t2_own", [own, t2w], F32, kind="Internal")
    t2_full = shared("t2_full", [nslots, t2w], F32)
    pool_in = nc.dram_tensor("pool_in", [1, hf], F32, kind="Internal")
    pool_out = shared("pool_out", [1, hf])
    scratch_d = nc.dram_tensor("scratch", [1, max(hf, 32)], F32,
                               kind="Internal")

    rg = [list(range(n_cores))]
    q_counter = [0]

    def gather(dst_tile_ap, table_ap, idx_ap):
        inst = nc.gpsimd.indirect_dma_start(
            out=dst_tile_ap, out_offset=None, in_=table_ap,
            in_offset=bass.IndirectOffsetOnAxis(ap=idx_ap, axis=0))
        if n_queues > 1:
            qi = q_counter[0] % n_queues
            q_counter[0] += 1
            inst.ins.queue = f"qPoolDynamic{qi or ''}"
        return inst

    with tile.TileContext(nc) as tc:
        with tc.tile_pool(name="const", bufs=1) as constp, \
             tc.tile_pool(name="cache", bufs=1) as cachep, \
             tc.tile_pool(name="src", bufs=2) as srcp, \
             tc.tile_pool(name="src3", bufs=3) as src3p, \
             tc.tile_pool(name="ohh", bufs=2) as ohhp, \
             tc.tile_pool(name="blk", bufs=2) as blkp, \
             tc.tile_pool(name="small", bufs=4) as smallp, \
             tc.tile_pool(name="fin", bufs=1) as finp, \
             tc.tile_pool(name="psA", bufs=2, space="PSUM") as psA, \
             tc.tile_pool(name="psB", bufs=1, space="PSUM") as psB, \
             tc.tile_pool(name="psS", bufs=2, space="PSUM") as psS, \
             tc.tile_pool(name="psP", bufs=1, space="PSUM") as psP:

            # ---------------- constants / caches
            iota_bf = constp.tile([128, 128], BF16)
            nc.gpsimd.iota(iota_bf[:], pattern=[[1, 128]], base=0,
                           channel_multiplier=0,
                           allow_small_or_imprecise_dtypes=True)
            iota_colf = constp.tile([128, 1], F32)
            nc.gpsimd.iota(iota_colf[:], pattern=[[0, 1]], base=0,
                           channel_multiplier=1,
                           allow_small_or_imprecise_dtypes=True)
            iota_col128 = constp.tile([128, 128], BF16)
            nc.gpsimd.iota(iota_col128[:], pattern=[[0, 128]], base=0,
                           channel_multiplier=1,
                           allow_small_or_imprecise_dtypes=True)
            ident_bf = constp.tile([128, 128], BF16)
            nc.vector.tensor_tensor(out=ident_bf[:], in0=iota_col128[:],
                                    in1=iota_bf[:], op=ALU.is_equal)
            ones_row = constp.tile([1, 128], BF16)
            nc.vector.memset(ones_row[:], 1.0)

            w1sb = constp.tile([f_in, d1], F32)
            nc.sync.dma_start(out=w1sb[:], in_=w1[:])
            b1sb = constp.tile([128, d1], F32)
            nc.sync.dma_start(out=b1sb[:], in_=b1r[:])
            w2sb = constp.tile([d1, d2], F32)
            nc.sync.dma_start(out=w2sb[:], in_=w2[:])
            b2sb = constp.tile([128, d2], F32)
            nc.sync.dma_start(out=b2sb[:], in_=b2r[:])
            wgsb = constp.tile([d2, hf], BF16)
            nc.sync.dma_start(out=wgsb[:], in_=wgbf[:])
            awadsb = constp.tile([d2, 2 * h_heads], BF16)
            nc.sync.dma_start(out=awadsb[:], in_=awad[:])
            bgsb = constp.tile([1, hf], BF16)
            nc.sync.dma_start(out=bgsb[:], in_=bgrow[:])

            dinv_all = cachep.tile([128, nblk], F32)
            nc.sync.dma_start(out=dinv_all[:], in_=dinv_in[:])
            vm_all = cachep.tile([128, nblk], BF16)
            nc.sync.dma_start(out=vm_all[:], in_=vm_in[:])
            dv_all = cachep.tile([128, nblk * tpb], F32)
            nc.sync.dma_start(out=dv_all[:], in_=dv_in[:])
            gidx_all = cachep.tile([128, nblk * tpb], I32)
            nc.sync.dma_start(out=gidx_all[:], in_=gidx[:])

            oh_all = cachep.tile([128, nblk * tpb * 128], BF16)
            t1c = cachep.tile([128, nblk * f_in], BF16)
            nc.vector.memset(t1c[:], 0.0)
            t2hc = cachep.tile([128, nblk * d2], BF16)
            aac = cachep.tile([128, nblk * 2 * h_heads], F32)  # als|ald own

            def _whole_body(_i=None):
                # ---------------- phase 1: GCN1 (host-pregathered messages)
                for b in range(nblk):
                    m1 = srcp.tile([128, tpb * f_in], BF16, tag="m1")
                    nc.sync.dma_start(
                        out=m1[:],
                        in_=msg1[:, b * tpb * f_in:(b + 1) * tpb * f_in])
                    agg = psA.tile([f_in, 128], F32, space="PSUM", tag="agg")
                    for t in range(tpb):
                        o = oh_all[:, (b * tpb + t) * 128:(b * tpb + t + 1) * 128]
                        nc.vector.tensor_scalar(
                            out=o, in0=iota_bf[:],
                            scalar1=dv_all[:, b * tpb + t:b * tpb + t + 1],
                            scalar2=None, op0=ALU.is_equal)
                        nc.tensor.matmul(
                            out=agg[:], lhsT=m1[:, t * f_in:(t + 1) * f_in],
                            rhs=o, start=(t == 0), stop=False)
                    xsb_b = smallp.tile([128, f_in], BF16, tag="xsbb")
                    nc.sync.dma_start(
                        out=xsb_b[:], in_=xsb[:, b * f_in:(b + 1) * f_in])
                    nc.tensor.matmul(
                        out=agg[:], lhsT=xsb_b[:],
                        rhs=ident_bf[:], start=False, stop=True)
                    aggs = blkp.tile([f_in, 128], F32, tag="aggs")
                    nc.scalar.activation(out=aggs[:], in_=agg[:],
                                         func=ACT.Copy)
                    h1ps = psS.tile([128, d1], F32, space="PSUM", tag="tr")
                    nc.tensor.matmul(out=h1ps[:], lhsT=aggs[:], rhs=w1sb[:],
                                     start=True, stop=True)
                    h1a = smallp.tile([128, d1], F32, tag="h1a")
                    nc.vector.tensor_scalar(
                        out=h1a[:], in0=h1ps[:],
                        scalar1=dinv_all[:, b:b + 1], scalar2=None,
                        op0=ALU.mult)
                    nc.vector.tensor_tensor(
                        out=h1a[:], in0=h1a[:],
                        in1=b1sb[:], op=ALU.add)
                    nc.scalar.activation(out=h1a[:], in_=h1a[:], func=ACT.Relu)
                    nc.vector.tensor_scalar(
                        out=t1c[:, b * f_in:b * f_in + d1], in0=h1a[:],
                        scalar1=dinv_all[:, b:b + 1], scalar2=None,
                        op0=ALU.mult)
                    nc.sync.dma_start(
                        out=t1_own[b * 128:(b + 1) * 128, :],
                        in_=t1c[:, b * f_in:(b + 1) * f_in])

                if timing_repeats:
                    nc.sync.dma_start(out=t1_full[:own, :], in_=t1_own[:])
                else:
                    nc.gpsimd.collective_compute(
                        "AllGather", ALU.bypass, replica_groups=rg,
                        ins=[t1_own[:]], outs=[t1_full[:]])

                # ---------------- phase 2: GCN2 -> t2 table [h2.bf16 | als]
                for b in range(nblk):
                    t1t = srcp.tile([128, tpb, f_in], BF16, tag="t1t")
                    for t in range(tpb):
                        gather(t1t[:, t, :], t1_full[:],
                               gidx_all[:, b * tpb + t:b * tpb + t + 1])
                    agg = psA.tile([f_in, 128], F32, space="PSUM", tag="agg")
                    for t in range(tpb):
                        o = oh_all[:, (b * tpb + t) * 128:(b * tpb + t + 1) * 128]
                        nc.tensor.matmul(out=agg[:], lhsT=t1t[:, t, :], rhs=o,
                                         start=(t == 0), stop=False)
                    nc.tensor.matmul(
                        out=agg[:], lhsT=t1c[:, b * f_in:(b + 1) * f_in],
                        rhs=ident_bf[:], start=False, stop=True)
                    aggs = blkp.tile([f_in, 128], F32, tag="aggs")
                    nc.scalar.activation(out=aggs[:], in_=agg[:],
                                         func=ACT.Copy)
                    h2ps = psS.tile([128, d2], F32, space="PSUM", tag="tr")
                    nc.tensor.matmul(out=h2ps[:], lhsT=aggs[:d1, :],
                                     rhs=w2sb[:], start=True, stop=True)
                    h2a = blkp.tile([128, d2], F32, tag="h2a")
                    nc.vector.tensor_scalar(
                        out=h2a[:], in0=h2ps[:],
                        scalar1=dinv_all[:, b:b + 1], scalar2=None,
                        op0=ALU.mult)
                    nc.vector.tensor_tensor(
                        out=h2a[:], in0=h2a[:],
                        in1=b2sb[:], op=ALU.add)
                    h2blk = t2hc[:, b * d2:(b + 1) * d2]
                    nc.scalar.activation(out=h2blk, in_=h2a[:], func=ACT.Relu)
                    # h2T (bf16) for als/ald
                    h2Tps = psS.tile([d2, 128], BF16, space="PSUM", tag="tr")
                    nc.tensor.transpose(out=h2Tps[:], in_=h2blk,
                                        identity=ident_bf[:])
                    h2T = blkp.tile([d2, 128], BF16, tag="h2T")
                    nc.vector.tensor_copy(out=h2T[:], in_=h2Tps[:])
                    aaps = psS.tile([128, 2 * h_heads], F32, space="PSUM",
                                    tag="tr")
                    nc.tensor.matmul(out=aaps[:], lhsT=h2T[:], rhs=awadsb[:],
                                     start=True, stop=True)
                    aa = aac[:, b * 2 * h_heads:(b + 1) * 2 * h_heads]
                    nc.vector.tensor_copy(out=aa, in_=aaps[:])
                    # table row: [h2 bf16 (64 f32 words) | als (8 f32)]
                    nc.sync.dma_start(
                        out=t2_own[b * 128:(b + 1) * 128, :d2 // 2],
                        in_=h2blk.bitcast(F32))
                    nc.sync.dma_start(
                        out=t2_own[b * 128:(b + 1) * 128, d2 // 2:],
                        in_=aa[:, :h_heads])

                if timing_repeats:
                    nc.sync.dma_start(out=t2_full[:own, :], in_=t2_own[:])
                else:
                    nc.gpsimd.collective_compute(
                        "AllGather", ALU.bypass, replica_groups=rg,
                        ins=[t2_own[:]], outs=[t2_full[:]])

                # ---------------- phase 3: GAT
                pool_ps0 = psP.tile([1, hf // 2], F32, space="PSUM",
                                    tag="pool0", name="pool_ps0")
                pool_ps1 = psP.tile([1, hf // 2], F32, space="PSUM",
                                    tag="pool1", name="pool_ps1")
                pool_ps = [pool_ps0, pool_ps1]
                for b in range(nblk):
                    t2t = src3p.tile([128, tpb, t2w], F32, tag="t2t")
                    for t in range(tpb):
                        gather(t2t[:, t, :], t2_full[:],
                               gidx_all[:, b * tpb + t:b * tpb + t + 1])
                    # ---- pass A: exts for all tiles
                    aldb = aac[:, b * 2 * h_heads + h_heads:
                               (b + 1) * 2 * h_heads]
                    alde_ps = psS.tile([128, tpb, h_heads], F32, space="PSUM",
                                       tag="tr")
                    aldb_bf = smallp.tile([128, h_heads], BF16,
                                          tag="aldbbf")
                    nc.vector.tensor_copy(out=aldb_bf[:], in_=aldb)
                    for t in range(tpb):
                        o = oh_all[:, (b * tpb + t) * 128:(b * tpb + t + 1) * 128]
                        ohT_ps = psS.tile([128, 128], BF16, space="PSUM",
                                          tag="tr")
                        nc.tensor.transpose(out=ohT_ps[:], in_=o,
                                            identity=ident_bf[:])
                        ohT = smallp.tile([128, 128], BF16, tag="ohT")
                        nc.scalar.activation(out=ohT[:], in_=ohT_ps[:],
                                             func=ACT.Copy)
                        nc.tensor.matmul(out=alde_ps[:, t, :], lhsT=ohT[:],
                                         rhs=aldb_bf[:], start=True, stop=True)
                    lg = blkp.tile([128, tpb, h_heads], F32, tag="lg")
                    nc.vector.tensor_tensor(
                        out=lg[:], in0=alde_ps[:],
                        in1=t2t[:, :, d2 // 2:], op=ALU.add)
                    lneg = blkp.tile([128, tpb, h_heads], F32, tag="lneg")
                    nc.vector.tensor_scalar(out=lneg[:], in0=lg[:],
                                            scalar1=0.0, scalar2=NEG_SLOPE,
                                            op0=ALU.min, op1=ALU.mult)
                    nc.vector.tensor_scalar(out=lg[:], in0=lg[:], scalar1=0.0,
                                            scalar2=None, op0=ALU.max)
                    nc.vector.tensor_tensor(out=lg[:], in0=lg[:], in1=lneg[:],
                                            op=ALU.add)
                    exts = blkp.tile([128, tpb, h_heads], F32, tag="exts")
                    nc.scalar.activation(out=exts[:], in_=lg[:], func=ACT.Exp)
                    exts_bf = blkp.tile([128, tpb, h_heads], BF16, tag="extsb")
                    nc.vector.tensor_copy(out=exts_bf[:], in_=exts[:])
                    # self logits
                    aa = aac[:, b * 2 * h_heads:(b + 1) * 2 * h_heads]
                    lgs = smallp.tile([128, h_heads], F32, tag="lgs")
                    nc.vector.tensor_tensor(out=lgs[:], in0=aa[:, :h_heads],
                                            in1=aa[:, h_heads:], op=ALU.add)
                    lnegs = smallp.tile([128, h_heads], F32, tag="lnegs")
                    nc.vector.tensor_scalar(out=lnegs[:], in0=lgs[:],
                                            scalar1=0.0, scalar2=NEG_SLOPE,
                                            op0=ALU.min, op1=ALU.mult)
                    nc.vector.tensor_scalar(out=lgs[:], in0=lgs[:],
                                            scalar1=0.0, scalar2=None,
                                            op0=ALU.max)
                    nc.vector.tensor_tensor(out=lgs[:], in0=lgs[:],
                                            in1=lnegs[:], op=ALU.add)
                    ex_self = smallp.tile([128, h_heads], F32, tag="exself")
                    nc.scalar.activation(out=ex_self[:], in_=lgs[:],
                                         func=ACT.Exp)
                    ex_self_bf = smallp.tile([128, h_heads], BF16,
                                             tag="exselfb")
                    nc.vector.tensor_copy(out=ex_self_bf[:], in_=ex_self[:])
                    # s per dst slot
                    s_ps = psS.tile([128, h_heads], F32, space="PSUM",
                                    tag="tr")
                    for t in range(tpb):
                        o = oh_all[:, (b * tpb + t) * 128:(b * tpb + t + 1) * 128]
                        nc.tensor.matmul(out=s_ps[:], lhsT=o,
                                         rhs=exts_bf[:, t, :],
                                         start=(t == 0), stop=False)
                    nc.tensor.matmul(out=s_ps[:], lhsT=ident_bf[:],
                                     rhs=ex_self_bf[:], start=False, stop=True)
                    rblk = smallp.tile([128, h_heads], F32, tag="rblk")
                    nc.vector.tensor_scalar(out=rblk[:], in0=s_ps[:],
                                            scalar1=EPS, scalar2=None,
                                            op0=ALU.add)
                    nc.vector.reciprocal(out=rblk[:], in_=rblk[:])
                    # ---- pass B: aggregate exts-weighted h2
                    agg3 = psB.tile([128, h_heads, 128], F32, space="PSUM",
                                    tag="agg3")
                    for t in range(tpb):
                        ohh = ohhp.tile([128, h_heads, 128], BF16, tag="ohh")
                        for hh in range(h_heads):
                            nc.vector.tensor_scalar(
                                out=ohh[:, hh, :], in0=iota_bf[:],
                                scalar1=dv_all[:, b * tpb + t:b * tpb + t + 1],
                                scalar2=exts[:, t, hh:hh + 1],
                                op0=ALU.is_equal, op1=ALU.mult)
                        h2src = t2t[:, t, :d2 // 2].bitcast(BF16)
                        for half in range(2):
                            nc.tensor.matmul(
                                out=agg3[:, half * 4:(half + 1) * 4, :],
                                lhsT=h2src,
                                rhs=ohh[:, half * 4:(half + 1) * 4, :],
                                start=(t == 0), stop=False,
                                skip_group_check=True)
                    # self contributions
                    h2self = t2hc[:, b * d2:(b + 1) * d2]
                    for hh in range(h_heads):
                        hsc = smallp.tile([128, d2], BF16, tag="hsc")
                        nc.vector.tensor_scalar(
                            out=hsc[:], in0=h2self,
                            scalar1=ex_self[:, hh:hh + 1], scalar2=None,
                            op0=ALU.mult)
                        nc.tensor.matmul(out=agg3[:, hh, :], lhsT=hsc[:],
                                         rhs=ident_bf[:], start=False,
                                         stop=(hh in (3, 7)),
                                         skip_group_check=True)
                    aggsb = blkp.tile([128, h_heads, 128], BF16, tag="agg3s")
                    nc.scalar.activation(out=aggsb[:], in_=agg3[:],
                                         func=ACT.Copy)
                    # og = agg @ Wg per head (+ bg), then r-scale + relu
                    og = psB.tile([128, hf], F32, space="PSUM", tag="agg3")
                    for half in range(2):
                        nc.tensor.matmul(
                            out=og[:, half * (hf // 2):(half + 1) * (hf // 2)],
                            lhsT=ones_row[:],
                            rhs=bgsb[:, half * (hf // 2):(half + 1) * (hf // 2)],
                            start=True, stop=False, skip_group_check=True)
                    for hh in range(h_heads):
                        nc.tensor.matmul(
                            out=og[:, hh * f_gat:(hh + 1) * f_gat],
                            lhsT=aggsb[:, hh, :],
                            rhs=wgsb[:, hh * f_gat:(hh + 1) * f_gat],
                            start=False, stop=(hh % 4 == 3),
                            skip_group_check=True)
                    gat = blkp.tile([128, hf], BF16, tag="gat")
                    nc.vector.tensor_tensor(
                        out=gat[:].rearrange("p (h f) -> p h f", h=h_heads),
                        in0=og[:].rearrange("p (h f) -> p h f", h=h_heads),
                        in1=rblk[:].rearrange("p (h o) -> p h o", o=1)
                            .to_broadcast([128, h_heads, f_gat]),
                        op=ALU.mult)
                    nc.scalar.activation(out=gat[:], in_=gat[:], func=ACT.Relu)
                    for half in range(2):
                        nc.tensor.matmul(
                            out=pool_ps[half][:],
                            lhsT=vm_all[:, b:b + 1],
                            rhs=gat[:, half * (hf // 2):(half + 1) * (hf // 2)],
                            start=(b == 0), stop=(b == nblk - 1))

                # ---------------- phase 4: AllReduce pooled, fc, softmax
                pooled = finp.tile([1, hf], F32, tag="pooled")
                nc.vector.tensor_copy(out=pooled[:, :hf // 2],
                                      in_=pool_ps[0][:])
                nc.vector.tensor_copy(out=pooled[:, hf // 2:],
                                      in_=pool_ps[1][:])
                nc.sync.dma_start(out=pool_in[:], in_=pooled[:1, :])
                if timing_repeats:
                    nc.sync.dma_start(out=pool_out[:], in_=pool_in[:])
                else:
                    nc.gpsimd.collective_compute(
                        "AllReduce", ALU.add, replica_groups=rg,
                        ins=[pool_in[:]], outs=[pool_out[:]])
                mean = pooled
                nc.sync.dma_start(out=mean[:], in_=pool_out[:])
                nc.vector.tensor_scalar(out=mean[:], in0=mean[:],
                                        scalar1=1.0 / n_real, scalar2=None,
                                        op0=ALU.mult)
                nc.sync.dma_start(out=scratch_d[0, :hf], in_=mean[:1, :])
                fc_ps = psS.tile([ncls, 1], F32, space="PSUM", tag="tr")
                n_chunks = hf // 128
                for ci in range(n_chunks):
                    mcol = smallp.tile([128, 1], F32, tag="mcol")
                    nc.sync.dma_start(
                        out=mcol[:],
                        in_=scratch_d[0, ci * 128:(ci + 1) * 128, None])
                    wfc_sb = smallp.tile([128, ncls], F32, tag="wfcsb")
                    nc.sync.dma_start(out=wfc_sb[:],
                                      in_=wfc[ci * 128:(ci + 1) * 128, :])
                    nc.tensor.matmul(out=fc_ps[:], lhsT=wfc_sb[:], rhs=mcol[:],
                                     start=(ci == 0), stop=(ci == n_chunks - 1))
                bfc_sb = smallp.tile([ncls, 1], F32, tag="bfcsb")
                nc.sync.dma_start(out=bfc_sb[:], in_=bfc[:])
                logit = smallp.tile([ncls, 1], F32, tag="logit")
                nc.vector.tensor_tensor(out=logit[:], in0=fc_ps[:],
                                        in1=bfc_sb[:], op=ALU.add)
                nc.sync.dma_start(out=scratch_d[0, :ncls, None],
                                  in_=logit[:, :1])
                lrow = smallp.tile([1, ncls], F32, tag="lrow")
                nc.sync.dma_start(out=lrow[:], in_=scratch_d[:1, :ncls])
                erow = smallp.tile([1, ncls], F32, tag="erow")
                nc.scalar.activation(out=erow[:], in_=lrow[:], func=ACT.Exp)
                ssum = smallp.tile([1, 1], F32, tag="ssum")
                nc.vector.reduce_sum(out=ssum[:], in_=erow[:], axis=AX.X)
                nc.vector.reciprocal(out=ssum[:], in_=ssum[:])
                nc.vector.tensor_tensor(
                    out=erow[:], in0=erow[:],
                    in1=ssum[:1, :1].to_broadcast([1, ncls]), op=ALU.mult)
                nc.sync.dma_start(out=out[None, :], in_=erow[:1, :])

            if timing_repeats > 0:
                with tc.For_i(0, timing_repeats, 1) as _i:
                    _whole_body(_i)
            else:
                _whole_body()

    nc.compile()
    return nc


def make_in_maps(per_core, w):
    maps = []
    for pc in per_core:
        m = dict(pc)
        m.update(w)
        maps.append(m)
    return maps


def weights_dict(W1, b1, W2, b2, Wg, a_src, a_dst, bg, Wfc, bfc):
    Wg = np.asarray(Wg, np.float32)
    a_src = np.asarray(a_src, np.float32)
    a_dst = np.asarray(a_dst, np.float32)
    H, F = a_src.shape
    aw = np.einsum("khf,hf->kh", Wg.reshape(Wg.shape[0], H, F), a_src)
    ad = np.einsum("khf,hf->kh", Wg.reshape(Wg.shape[0], H, F), a_dst)
    return dict(
        w1=np.asarray(W1, np.float32),
        b1r=np.tile(np.asarray(b1, np.float32).reshape(1, -1), (128, 1)),
        w2=np.asarray(W2, np.float32),
        b2r=np.tile(np.asarray(b2, np.float32).reshape(1, -1), (128, 1)),
        wgbf=_bf16(Wg),
        awad=_bf16(np.concatenate([aw, ad], axis=1)),
        bgrow=_bf16(np.asarray(bg, np.float32).reshape(1, -1)),
        wfc=np.asarray(Wfc, np.float32),
        bfc=np.asarray(bfc, np.float32).reshape(-1, 1),
    )


# ------------------------------------------------------------ harness entry

_CACHE = {}


def kernel(**inputs):
    """Full-input entry: shards across 8 trn2 cores internally."""
    x = np.asarray(inputs["x"], dtype=np.float32)
    edge_index = np.asarray(inputs["edge_index"])
    n_cores = 8

    per_core, meta = prep(x, edge_index, n_cores=n_cores)
    shapes = dict(f_in=128, d1=64, d2=128, h=8, f=128, ncls=10)

    key = (meta["nblk"], meta["tpb"], meta["nslots"], meta["n"])
    if key in _CACHE:
        nc = _CACHE[key]
    else:
        nc = build(meta, shapes, n_cores=n_cores, n_queues=4)
        _CACHE[key] = nc

    wd = weights_dict(inputs["W1"], inputs["b1"], inputs["W2"],
                      inputs["b2"], inputs["Wg"], inputs["a_src"],
                      inputs["a_dst"], inputs["bg"], inputs["Wfc"],
                      inputs["bfc"])
    in_maps = make_in_maps(per_core, wd)

    from concourse.bass_utils import run_bass_kernel_spmd
    res = run_bass_kernel_spmd(nc, in_maps, core_ids=list(range(n_cores)))
    return np.asarray(res.results[0]["out"], dtype=np.float32)
